# revision 1
# baseline (speedup 1.0000x reference)
"""Sliding-window KV cache append on 8 trn2 NeuronCores.

new_k = concat(cache_k, k, axis=2)[:, :, -4096:, :]  (same for v)
      = cache_k shifted left by 16 seq positions with k appended.

Pure memory movement. Sharding: head-parallel — 32 heads split 4 per core,
no cross-core communication. Per core the kernel is DRAM->DRAM DMA copies:
for each (batch, head): a contiguous ~2 MiB copy of the cache tail into
rows 0..4079 of the output, plus an 8 KiB copy of the new rows into the
output tail. k-tensor copies issue on the sync engine (HWDGE), v-tensor
copies on the scalar engine (HWDGE) so the two descriptor rings run in
parallel.
"""

import numpy as np

import concourse.bass as bass
import concourse.mybir as mybir
from concourse.bass_utils import run_bass_kernel_spmd

B = 2          # batch
H = 32         # total heads
L = 4096       # cache length (MAX_LEN)
D = 128        # head dim
NEW = 16       # appended rows
N_CORES = 8
HPC = H // N_CORES   # heads per core
KEEP = L - NEW       # rows kept from the old cache

_NC = None


def _build_nc() -> bass.Bass:
    nc = bass.Bass(enable_partition_id=False)
    f32 = mybir.dt.float32

    ck = nc.declare_dram_parameter("cache_k", [B, HPC, L, D], f32, isOutput=False)
    cv = nc.declare_dram_parameter("cache_v", [B, HPC, L, D], f32, isOutput=False)
    kn = nc.declare_dram_parameter("k", [B, HPC, NEW, D], f32, isOutput=False)
    vn = nc.declare_dram_parameter("v", [B, HPC, NEW, D], f32, isOutput=False)
    ok = nc.declare_dram_parameter("out_k", [B, HPC, L, D], f32, isOutput=True)
    ov = nc.declare_dram_parameter("out_v", [B, HPC, L, D], f32, isOutput=True)

    # One dma_start per contiguous ~2 MiB block: a single-dim AP is split into
    # <=64 KiB descriptors sprayed across all 16 SDMA engines (the spray
    # follows the slowest AP dim, so fusing blocks into one strided dma_start
    # would cut the spray to 8 engines and cost ~40% bandwidth).
    with (
        nc.Block(no_gpsimd_drain=True) as block,
        nc.semaphore("sem_k") as sem_k,
        nc.semaphore("sem_v") as sem_v,
    ):

        @block.sync
        def _(sync: bass.BassEngine):
            # new rows first: the small strided DMA (8 KiB/descriptor) rides
            # the engine-ramp window instead of trailing the big copies
            sync.dma_start(out=ok[:, :, KEEP:L, :], in_=kn[:]).then_inc(sem_k, 16)
            n = 1
            for b in range(B):
                for h in range(HPC):
                    sync.dma_start(
                        out=ok[b, h, 0:KEEP, :], in_=ck[b, h, NEW:L, :]
                    ).then_inc(sem_k, 16)
                    n += 1
            sync.wait_ge(sem_k, 16 * n)

        @block.scalar
        def _(scalar: bass.BassEngine):
            scalar.dma_start(out=ov[:, :, KEEP:L, :], in_=vn[:]).then_inc(sem_v, 16)
            n = 1
            for b in range(B):
                for h in range(HPC):
                    scalar.dma_start(
                        out=ov[b, h, 0:KEEP, :], in_=cv[b, h, NEW:L, :]
                    ).then_inc(sem_v, 16)
                    n += 1
            scalar.wait_ge(sem_v, 16 * n)

    return nc


def _get_nc() -> bass.Bass:
    global _NC
    if _NC is None:
        _NC = _build_nc()
    return _NC


def _in_maps(inputs: dict) -> list[dict]:
    cache_k = np.asarray(inputs["cache_k"], dtype=np.float32)
    cache_v = np.asarray(inputs["cache_v"], dtype=np.float32)
    k = np.asarray(inputs["k"], dtype=np.float32)
    v = np.asarray(inputs["v"], dtype=np.float32)
    maps = []
    for c in range(N_CORES):
        sl = slice(c * HPC, (c + 1) * HPC)
        maps.append(
            {
                "cache_k": np.ascontiguousarray(cache_k[:, sl]),
                "cache_v": np.ascontiguousarray(cache_v[:, sl]),
                "k": np.ascontiguousarray(k[:, sl]),
                "v": np.ascontiguousarray(v[:, sl]),
            }
        )
    return maps


def _gather(results: list[dict]) -> tuple[np.ndarray, np.ndarray]:
    new_k = np.concatenate([results[c]["out_k"] for c in range(N_CORES)], axis=1)
    new_v = np.concatenate([results[c]["out_v"] for c in range(N_CORES)], axis=1)
    return new_k, new_v


def kernel_traced(inputs: dict, **kwargs):
    """Run and also return the BassKernelResults (for profiling from test.py)."""
    res = run_bass_kernel_spmd(
        _get_nc(), _in_maps(inputs), list(range(N_CORES)), **kwargs
    )
    return _gather(res.results), res


def kernel(**inputs) -> tuple[np.ndarray, np.ndarray]:
    out, _ = kernel_traced(inputs)
    return out

